# revision 15
# baseline (speedup 1.0000x reference)
"""GAT layer (dense-adj variant) on 8 Trainium2 NeuronCores.

Strategy: row-parallel over destination nodes. Each core owns R=1024 rows of
the NxN score matrix / output; h (=x@fc_w+fc_b) is computed replicated on
every core. Scores are built in transposed layout [j (src) on partitions,
i (dest) on free] so the final attn@h matmul contracts j on partitions
directly. The softmax denominator Z rides along as column 256 of the moving
operand (h_aug's ones column), accumulated by the same matmuls as out.

Math (exact rank-1 decomposition of the reference):
  src = x@(fc_w@a_src) + (fc_b@a_src + attn_b)
  dst = x@(fc_w@a_dst) + (fc_b@a_dst)
  E[j,i] = exp(leaky_relu_{0.01}(src_i+dst_j) * adj[i,j])       (adj in {0,1})
  out[i,:] = (sum_j E[j,i] * h[j,:]) / (sum_j E[j,i])

Engine-level layout decisions (from NTFF traces):
- All elementwise data is bf16 (DVE 2x/4x modes; softmax rows are dominated
  by the 8191 exact exp(0)=1 non-edge terms per row, so bf16 score noise on
  the ~1% edges is invisible: emulated end-to-end rel err 3.2e-3 vs 3.0e-3
  for an all-f32 elementwise path).
- Per j-strip the E computation alternates between two equivalent forms to
  balance ScalarE vs VectorE:
    S1 (ACT-heavy): l = Prelu(src+dst) [ACT], za = l*adj [DVE], E = exp(za) [ACT]
    S2 (DVE-heavy): zb = src+dst [DVE], za = zb*adj [DVE], e1 = exp(za) [ACT],
                    t = 1+0.01*za [DVE], E = max(e1, t) [DVE]
  S2 uses exp(leaky(z)*adj) = exp(leaky(z*adj)) = max(exp(za), exp(0.01*za))
  with exp(0.01*za) ~ 1+0.01*za (error < 2e-3, exact at za=0 so non-edges
  stay exactly 1). Prelu/Exp share one ACT table set: no table reloads.
- fc_b/ones/b_dst enter h_aug through a 5th K=1 matmul (ones-row x fcb_row),
  so the PSUM->SBUF hop is a plain 2x-mode copy on DVE.
- One 8-bank PSUM pool: acc0..acc5 accumulate i-tiles 0..5 starting at strip
  0 (interleaved with phase B in the PE stream); banks 6/7 double as phase
  A/B scratch, so i-tiles 6/7 accumulate in a short tail after B finishes.
- Engines execute their instruction streams IN ORDER, so phase-B and phase-C
  work is emitted interleaved per 8-strip chunk; emitting all of B first
  starves ScalarE/TensorE until B completes.
"""

import numpy as np
import ml_dtypes

N = 8192
IN_DIM = 512
OUT_DIM = 256
NCORES = 8
R = N // NCORES  # 1024 rows per core
KT = IN_DIM // 128  # 4 k-tiles
JT = N // 128  # 64 j-strips
IT = R // 128  # 8 i-tiles per core
HA = OUT_DIM + 1  # h_aug matmul width (h | ones)
HS = OUT_DIM + 2  # h_sb slot width  (h | ones | dst)
GC = 8  # strips per emission chunk
N_EARLY = 6  # i-tiles accumulating from strip 0 (banks 0..5)

bf16 = ml_dtypes.bfloat16

_cache = {}

# Number of j-strips handled with the ACT-heavy form (S1); rest are S2.
N_S1 = 35


def _build():
    import concourse.tile as tile
    from concourse import bacc, mybir

    AF = mybir.ActivationFunctionType
    ALU = mybir.AluOpType
    f32 = mybir.dt.float32
    bft = mybir.dt.bfloat16

    s1_strips = set(np.linspace(0, JT - 1, N_S1).astype(int).tolist())

    nc = bacc.Bacc("TRN2", target_bir_lowering=False, debug=False)

    adjT_d = nc.dram_tensor("adjT", [N, R], bft, kind="ExternalInput").ap()
    xT_d = nc.dram_tensor("xT", [IN_DIM, N], bft, kind="ExternalInput").ap()
    xTi_d = nc.dram_tensor("xTi", [IN_DIM, R], bft, kind="ExternalInput").ap()
    # rhs_aug columns: [fc_w (256) | zeros (1) | w_dst (1)]
    rhs_aug_d = nc.dram_tensor("rhs_aug", [IN_DIM, HS], bft, kind="ExternalInput").ap()
    # fcb_row: [fc_b (256) | 1.0 | b_dst] applied via a K=1 matmul
    fcb_row_d = nc.dram_tensor("fcb_row", [1, HS], bft, kind="ExternalInput").ap()
    w_src_rep_d = nc.dram_tensor("w_src_rep", [IN_DIM, 128], bft, kind="ExternalInput").ap()
    src_bias_d = nc.dram_tensor("src_bias", [128, 1], f32, kind="ExternalInput").ap()
    out_d = nc.dram_tensor("out", [R, OUT_DIM], f32, kind="ExternalOutput").ap()

    with tile.TileContext(nc) as tc:
        with (
            tc.tile_pool(name="const", bufs=1) as cpool,
            tc.tile_pool(name="hpool", bufs=1) as hpool,
            tc.tile_pool(name="xstream", bufs=8) as xpool,
            tc.tile_pool(name="astream", bufs=8) as apool,
            tc.tile_pool(name="work", bufs=3) as wpool,
            tc.tile_pool(name="estream", bufs=24) as epool,
            tc.tile_pool(name="opool", bufs=2) as opool,
            tc.tile_pool(name="psum", bufs=1, space="PSUM") as pspool,
        ):
            # ---- constants ----
            rhs_aug_sb = cpool.tile([128, KT * HS], bft)
            nc.sync.dma_start(
                rhs_aug_sb[:].rearrange("p (k n) -> p k n", k=KT),
                rhs_aug_d.rearrange("(k p) n -> p k n", p=128),
            )
            fcb_row_sb = cpool.tile([1, HS], bft)
            nc.sync.dma_start(fcb_row_sb[:], fcb_row_d)
            ones_row = cpool.tile([1, 128], bft)
            nc.vector.memset(ones_row[:], 1.0)
            w_src_sb = cpool.tile([128, KT * 128], bft)
            nc.sync.dma_start(
                w_src_sb[:].rearrange("p (k n) -> p k n", k=KT),
                w_src_rep_d.rearrange("(k p) n -> p k n", p=128),
            )
            xTi_sb = cpool.tile([128, KT * R], bft)
            nc.sync.dma_start(
                xTi_sb[:].rearrange("p (k n) -> p k n", k=KT),
                xTi_d.rearrange("(k p) n -> p k n", p=128),
            )
            src_bias_sb = cpool.tile([128, 1], f32)
            nc.sync.dma_start(src_bias_sb[:], src_bias_d)

            src_rep = cpool.tile([128, R], bft)
            h_sb = hpool.tile([128, JT * HS], bft)
            dst_sb = cpool.tile([128, JT], f32)
            e_strips = [None] * JT

            # acc0..acc5: i-tiles 0..5, accumulating from strip 0.
            # acc6/acc7: phase A/B scratch first, then i-tiles 6/7 in a tail.
            acc = [
                pspool.tile([128, 512], f32, name=f"acc{b}", tag=f"acc{b}")
                for b in range(8)
            ]

            def c_elementwise(jt):
                # E[j,i] strip for one 128-node j block (see module docstring)
                adjt = apool.tile([128, R], bft, name="adjt")
                nc.sync.dma_start(adjt[:], adjT_d[jt * 128 : (jt + 1) * 128, :])
                dst_j = dst_sb[:, jt : jt + 1]  # f32 [128,1]
                e = epool.tile([128, R], bft, name="e")
                if jt in s1_strips:
                    l = wpool.tile([128, R], bft, name="l", tag="l")
                    nc.scalar.activation(
                        l[:], src_rep[:], AF.Prelu, bias=dst_j, alpha=0.01,
                    )
                    za = wpool.tile([128, R], bft, name="za", tag="za")
                    nc.vector.tensor_mul(za[:], l[:], adjt[:])
                    nc.scalar.activation(e[:], za[:], AF.Exp)
                else:
                    zb = wpool.tile([128, R], bft, name="zb", tag="zb")
                    nc.vector.tensor_scalar_add(zb[:], src_rep[:], dst_j)
                    za = wpool.tile([128, R], bft, name="za", tag="za")
                    nc.vector.tensor_mul(za[:], zb[:], adjt[:])
                    e1 = wpool.tile([128, R], bft, name="e1", tag="e1")
                    nc.scalar.activation(e1[:], za[:], AF.Exp)
                    t = wpool.tile([128, R], bft, name="t", tag="t")
                    nc.vector.tensor_scalar(
                        t[:], za[:], 0.01, 1.0, ALU.mult, ALU.add,
                    )
                    nc.vector.tensor_max(e[:], e1[:], t[:])
                e_strips[jt] = e

            def c_matmuls(jt, its):
                e = e_strips[jt]
                hj = h_sb[:, jt * HS : jt * HS + HA]
                for it in its:
                    nc.tensor.matmul(
                        acc[it][:, 0:HA],
                        e[:, it * 128 : (it + 1) * 128],
                        hj,
                        start=(jt == 0),
                        stop=(jt == JT - 1),
                    )

            # ---- Phase A: src_rep[p, f] = src[i0+f] for all p ----
            for ch in range(R // 512):
                ps = acc[6 + ch]
                for kt in range(KT):
                    nc.tensor.matmul(
                        ps[:],
                        w_src_sb[:, kt * 128 : (kt + 1) * 128],
                        xTi_sb[:, kt * R + ch * 512 : kt * R + (ch + 1) * 512],
                        start=(kt == 0),
                        stop=(kt == KT - 1),
                    )
                nc.scalar.activation(
                    src_rep[:, ch * 512 : (ch + 1) * 512], ps[:], AF.Identity,
                    bias=src_bias_sb[:],
                )

            # ---- Phases B + C interleaved per chunk ----
            for jt in range(JT):
                xTj = xpool.tile([128, KT * 128], bft)
                nc.sync.dma_start(
                    xTj[:].rearrange("p (k n) -> p k n", k=KT),
                    xT_d[:, jt * 128 : (jt + 1) * 128].rearrange(
                        "(k p) n -> p k n", p=128
                    ),
                )
                ps = acc[6 + jt % 2]
                for kt in range(KT):
                    nc.tensor.matmul(
                        ps[:, 0:HS],
                        xTj[:, kt * 128 : (kt + 1) * 128],
                        rhs_aug_sb[:, kt * HS : (kt + 1) * HS],
                        start=(kt == 0),
                        stop=False,
                    )
                nc.tensor.matmul(
                    ps[:, 0:HS], ones_row[:], fcb_row_sb[:], start=False, stop=True,
                )
                # slot: [h+fc_b (256) | 1.0 | dst+b_dst], one 2x-mode copy
                nc.vector.tensor_copy(
                    h_sb[:, jt * HS : (jt + 1) * HS], ps[:, 0:HS],
                )
                if jt % GC == GC - 1:
                    g = jt // GC
                    nc.vector.tensor_copy(
                        dst_sb[:, g * GC : (g + 1) * GC],
                        h_sb[:, g * GC * HS : (g + 1) * GC * HS].rearrange(
                            "p (j s) -> p j s", s=HS
                        )[:, :, HS - 1 : HS],
                    )
                    for s_jt in range(g * GC, (g + 1) * GC):
                        c_elementwise(s_jt)
                        c_matmuls(s_jt, range(N_EARLY))

            # ---- tail: i-tiles 6/7 (banks freed once B finished) ----
            for jt in range(JT):
                c_matmuls(jt, range(N_EARLY, IT))

            # ---- Phase D: normalize rows (col 256 = Z) and store ----
            for it in range(IT):
                rz = opool.tile([128, 1], f32, tag="rz")
                nc.vector.reciprocal(rz[:], acc[it][:, OUT_DIM : OUT_DIM + 1])
                o = opool.tile([128, OUT_DIM], f32, tag="o")
                nc.vector.tensor_scalar_mul(o[:], acc[it][:, 0:OUT_DIM], rz[:])
                nc.sync.dma_start(out_d[it * 128 : (it + 1) * 128, :], o[:])

    nc.compile()
    return nc


def _prep_inputs(adj, x, fc_w, fc_b, attn_w, attn_b):
    fc_w = np.asarray(fc_w, np.float32)
    fc_b = np.asarray(fc_b, np.float32)
    attn_w = np.asarray(attn_w, np.float32)
    a_src = fc_w @ attn_w[:OUT_DIM]
    a_dst = fc_w @ attn_w[OUT_DIM:]
    b_src = float(fc_b @ attn_w[:OUT_DIM]) + float(attn_b)
    b_dst = float(fc_b @ attn_w[OUT_DIM:])

    xT = np.ascontiguousarray(np.asarray(x, np.float32).T).astype(bf16)
    adjT = np.asarray(adj, np.float32).astype(bf16).T  # [N (src j), N (dest i)]
    rhs_aug = np.concatenate(
        [fc_w, np.zeros((IN_DIM, 1), np.float32), a_dst[:, None]], axis=1
    ).astype(bf16)
    fcb_row = np.concatenate([fc_b, [1.0], [b_dst]]).astype(bf16)[None, :]
    w_src_rep = np.tile(a_src[:, None], (1, 128)).astype(bf16)
    src_bias = np.full((128, 1), b_src, np.float32)

    in_maps = []
    for c in range(NCORES):
        in_maps.append(
            {
                "adjT": np.ascontiguousarray(adjT[:, c * R : (c + 1) * R]),
                "xT": xT,
                "xTi": np.ascontiguousarray(xT[:, c * R : (c + 1) * R]),
                "rhs_aug": rhs_aug,
                "fcb_row": fcb_row,
                "w_src_rep": w_src_rep,
                "src_bias": src_bias,
            }
        )
    return in_maps


def kernel(adj, x, fc_w, fc_b, attn_w, attn_b, _trace=False, _tmpdir=None):
    from concourse import bass_utils

    if "nc" not in _cache:
        _cache["nc"] = _build()
    nc = _cache["nc"]
    in_maps = _prep_inputs(adj, x, fc_w, fc_b, attn_w, attn_b)
    res = bass_utils.run_bass_kernel_spmd(
        nc,
        in_maps,
        core_ids=list(range(NCORES)),
        trace=_trace,
        **({"tmpdir": _tmpdir} if _tmpdir else {}),
    )
    out = np.concatenate([res.results[c]["out"] for c in range(NCORES)], axis=0)
    if _trace:
        _cache["last_exec_time_ns"] = res.exec_time_ns
        _cache["last_profile_json"] = res.profile_json
    return out


# revision 16
# speedup vs baseline: 1.0214x; 1.0214x over previous
"""GAT layer (dense-adj variant) on 8 Trainium2 NeuronCores.

Strategy: row-parallel over destination nodes. Each core owns R=1024 rows of
the NxN score matrix / output; h (=x@fc_w+fc_b) is computed replicated on
every core. Scores are built in transposed layout [j (src) on partitions,
i (dest) on free] so the final attn@h matmul contracts j on partitions
directly. The softmax denominator Z rides along as column 256 of the moving
operand (h_aug's ones column), accumulated by the same matmuls as out.

Math (exact rank-1 decomposition of the reference):
  src = x@(fc_w@a_src) + (fc_b@a_src + attn_b)
  dst = x@(fc_w@a_dst) + (fc_b@a_dst)
  E[j,i] = exp(leaky_relu_{0.01}(src_i+dst_j) * adj[i,j])       (adj in {0,1})
  out[i,:] = (sum_j E[j,i] * h[j,:]) / (sum_j E[j,i])

Engine-level layout decisions (from NTFF traces):
- All elementwise data is bf16 (DVE 2x/4x modes; softmax rows are dominated
  by the 8191 exact exp(0)=1 non-edge terms per row, so bf16 score noise on
  the ~1% edges is invisible: emulated end-to-end rel err 3.2e-3 vs 3.0e-3
  for an all-f32 elementwise path).
- Per j-strip the E computation alternates between two equivalent forms to
  balance ScalarE vs VectorE:
    S1 (ACT-heavy): l = Prelu(src+dst) [ACT], za = l*adj [DVE], E = exp(za) [ACT]
    S2 (DVE-heavy): zb = src+dst [DVE], za = zb*adj [DVE], e1 = exp(za) [ACT],
                    t = 1+0.01*za [DVE], E = max(e1, t) [DVE]
  S2 uses exp(leaky(z)*adj) = exp(leaky(z*adj)) = max(exp(za), exp(0.01*za))
  with exp(0.01*za) ~ 1+0.01*za (error < 2e-3, exact at za=0 so non-edges
  stay exactly 1). Prelu/Exp share one ACT table set: no table reloads.
- fc_b/ones/b_dst enter h_aug through a 5th K=1 matmul (ones-row x fcb_row),
  so the PSUM->SBUF hop is a plain 2x-mode copy on DVE.
- One 8-bank PSUM pool: acc0..acc5 accumulate i-tiles 0..5 starting at strip
  0 (interleaved with phase B in the PE stream); banks 6/7 double as phase
  A/B scratch, so i-tiles 6/7 accumulate in a short tail after B finishes.
- Engines execute their instruction streams IN ORDER, so phase-B and phase-C
  work is emitted interleaved per 8-strip chunk; emitting all of B first
  starves ScalarE/TensorE until B completes.
"""

import numpy as np
import ml_dtypes

N = 8192
IN_DIM = 512
OUT_DIM = 256
NCORES = 8
R = N // NCORES  # 1024 rows per core
KT = IN_DIM // 128  # 4 k-tiles
JT = N // 128  # 64 j-strips
IT = R // 128  # 8 i-tiles per core
HA = OUT_DIM + 1  # h_aug matmul width (h | ones)
HS = OUT_DIM + 2  # h_sb slot width  (h | ones | dst)
GC = 8  # strips per emission chunk
N_EARLY = 6  # i-tiles accumulating from strip 0 (banks 0..5)

bf16 = ml_dtypes.bfloat16

_cache = {}

# Number of j-strips handled with the ACT-heavy form (S1); rest are S2.
N_S1 = 35


def _build():
    import concourse.tile as tile
    from concourse import bacc, mybir

    AF = mybir.ActivationFunctionType
    ALU = mybir.AluOpType
    f32 = mybir.dt.float32
    bft = mybir.dt.bfloat16

    s1_strips = set(np.linspace(0, JT - 1, N_S1).astype(int).tolist())

    nc = bacc.Bacc("TRN2", target_bir_lowering=False, debug=False)

    adjT_d = nc.dram_tensor("adjT", [N, R], bft, kind="ExternalInput").ap()
    xT_d = nc.dram_tensor("xT", [IN_DIM, N], bft, kind="ExternalInput").ap()
    xTi_d = nc.dram_tensor("xTi", [IN_DIM, R], bft, kind="ExternalInput").ap()
    # rhs_aug columns: [fc_w (256) | zeros (1) | w_dst (1)]
    rhs_aug_d = nc.dram_tensor("rhs_aug", [IN_DIM, HS], bft, kind="ExternalInput").ap()
    # fcb_row: [fc_b (256) | 1.0 | b_dst] applied via a K=1 matmul
    fcb_row_d = nc.dram_tensor("fcb_row", [1, HS], bft, kind="ExternalInput").ap()
    w_src_rep_d = nc.dram_tensor("w_src_rep", [IN_DIM, 128], bft, kind="ExternalInput").ap()
    src_bias_d = nc.dram_tensor("src_bias", [128, 1], f32, kind="ExternalInput").ap()
    out_d = nc.dram_tensor("out", [R, OUT_DIM], f32, kind="ExternalOutput").ap()

    with tile.TileContext(nc) as tc:
        with (
            tc.tile_pool(name="const", bufs=1) as cpool,
            tc.tile_pool(name="hpool", bufs=1) as hpool,
            tc.tile_pool(name="xstream", bufs=8) as xpool,
            tc.tile_pool(name="astream", bufs=8) as apool,
            tc.tile_pool(name="work", bufs=3) as wpool,
            tc.tile_pool(name="estream", bufs=24) as epool,
            tc.tile_pool(name="opool", bufs=2) as opool,
            tc.tile_pool(name="psum", bufs=1, space="PSUM") as pspool,
        ):
            # ---- constants ----
            rhs_aug_sb = cpool.tile([128, KT * HS], bft)
            nc.sync.dma_start(
                rhs_aug_sb[:].rearrange("p (k n) -> p k n", k=KT),
                rhs_aug_d.rearrange("(k p) n -> p k n", p=128),
            )
            fcb_row_sb = cpool.tile([1, HS], bft)
            nc.sync.dma_start(fcb_row_sb[:], fcb_row_d)
            ones_row = cpool.tile([1, 128], bft)
            nc.vector.memset(ones_row[:], 1.0)
            w_src_sb = cpool.tile([128, KT * 128], bft)
            nc.sync.dma_start(
                w_src_sb[:].rearrange("p (k n) -> p k n", k=KT),
                w_src_rep_d.rearrange("(k p) n -> p k n", p=128),
            )
            xTi_sb = cpool.tile([128, KT * R], bft)
            nc.sync.dma_start(
                xTi_sb[:].rearrange("p (k n) -> p k n", k=KT),
                xTi_d.rearrange("(k p) n -> p k n", p=128),
            )
            src_bias_sb = cpool.tile([128, 1], f32)
            nc.sync.dma_start(src_bias_sb[:], src_bias_d)

            src_rep = cpool.tile([128, R], bft)
            h_sb = hpool.tile([128, JT * HS], bft)
            dst_sb = cpool.tile([128, JT], f32)
            e_strips = [None] * JT

            # acc0..acc5: i-tiles 0..5, accumulating from strip 0.
            # acc6/acc7: phase A/B scratch first, then i-tiles 6/7 in a tail.
            acc = [
                pspool.tile([128, 512], f32, name=f"acc{b}", tag=f"acc{b}")
                for b in range(8)
            ]

            def c_elementwise(jt):
                # E[j,i] strip for one 128-node j block (see module docstring)
                adjt = apool.tile([128, R], bft, name="adjt")
                nc.sync.dma_start(adjt[:], adjT_d[jt * 128 : (jt + 1) * 128, :])
                dst_j = dst_sb[:, jt : jt + 1]  # f32 [128,1]
                e = epool.tile([128, R], bft, name="e")
                if jt in s1_strips:
                    l = wpool.tile([128, R], bft, name="l", tag="l")
                    nc.scalar.activation(
                        l[:], src_rep[:], AF.Prelu, bias=dst_j, alpha=0.01,
                    )
                    za = wpool.tile([128, R], bft, name="za", tag="za")
                    nc.vector.tensor_mul(za[:], l[:], adjt[:])
                    nc.scalar.activation(e[:], za[:], AF.Exp)
                else:
                    zb = wpool.tile([128, R], bft, name="zb", tag="zb")
                    nc.vector.tensor_scalar_add(zb[:], src_rep[:], dst_j)
                    za = wpool.tile([128, R], bft, name="za", tag="za")
                    nc.vector.tensor_mul(za[:], zb[:], adjt[:])
                    e1 = wpool.tile([128, R], bft, name="e1", tag="e1")
                    nc.scalar.activation(e1[:], za[:], AF.Exp)
                    t = wpool.tile([128, R], bft, name="t", tag="t")
                    nc.vector.tensor_scalar(
                        t[:], za[:], 0.01, 1.0, ALU.mult, ALU.add,
                    )
                    nc.vector.tensor_max(e[:], e1[:], t[:])
                e_strips[jt] = e

            def c_matmuls(jt, its):
                e = e_strips[jt]
                hj = h_sb[:, jt * HS : jt * HS + HA]
                for it in its:
                    nc.tensor.matmul(
                        acc[it][:, 0:HA],
                        e[:, it * 128 : (it + 1) * 128],
                        hj,
                        start=(jt == 0),
                        stop=(jt == JT - 1),
                    )

            # ---- Phase A: src_rep[p, f] = src[i0+f] for all p ----
            for ch in range(R // 512):
                ps = acc[6 + ch]
                for kt in range(KT):
                    nc.tensor.matmul(
                        ps[:],
                        w_src_sb[:, kt * 128 : (kt + 1) * 128],
                        xTi_sb[:, kt * R + ch * 512 : kt * R + (ch + 1) * 512],
                        start=(kt == 0),
                        stop=(kt == KT - 1),
                    )
                nc.scalar.activation(
                    src_rep[:, ch * 512 : (ch + 1) * 512], ps[:], AF.Identity,
                    bias=src_bias_sb[:],
                )

            # ---- Phases B + C interleaved per chunk ----
            for jt in range(JT):
                xTj = xpool.tile([128, KT * 128], bft)
                nc.sync.dma_start(
                    xTj[:].rearrange("p (k n) -> p k n", k=KT),
                    xT_d[:, jt * 128 : (jt + 1) * 128].rearrange(
                        "(k p) n -> p k n", p=128
                    ),
                )
                ps = acc[6 + jt % 2]
                for kt in range(KT):
                    nc.tensor.matmul(
                        ps[:, 0:HS],
                        xTj[:, kt * 128 : (kt + 1) * 128],
                        rhs_aug_sb[:, kt * HS : (kt + 1) * HS],
                        start=(kt == 0),
                        stop=False,
                    )
                nc.tensor.matmul(
                    ps[:, 0:HS], ones_row[:], fcb_row_sb[:], start=False, stop=True,
                )
                # slot: [h+fc_b (256) | 1.0 | dst+b_dst], one 2x-mode copy
                nc.vector.tensor_copy(
                    h_sb[:, jt * HS : (jt + 1) * HS], ps[:, 0:HS],
                )
                if jt % GC == GC - 1:
                    g = jt // GC
                    nc.vector.tensor_copy(
                        dst_sb[:, g * GC : (g + 1) * GC],
                        h_sb[:, g * GC * HS : (g + 1) * GC * HS].rearrange(
                            "p (j s) -> p j s", s=HS
                        )[:, :, HS - 1 : HS],
                    )
                    for s_jt in range(g * GC, (g + 1) * GC):
                        c_elementwise(s_jt)
                    # matmuls lag two chunks behind elementwise so the
                    # in-order PE stream never blocks on an unfinished e strip
                    if g >= 2:
                        for s_jt in range((g - 2) * GC, (g - 1) * GC):
                            c_matmuls(s_jt, range(N_EARLY))

            # ---- flush the lagged chunks, then i-tiles 6/7 ----
            for jt in range((JT // GC - 2) * GC, JT):
                c_matmuls(jt, range(N_EARLY))
            for jt in range(JT):
                c_matmuls(jt, range(N_EARLY, IT))

            # ---- Phase D: normalize rows (col 256 = Z) and store ----
            for it in range(IT):
                rz = opool.tile([128, 1], f32, tag="rz")
                nc.vector.reciprocal(rz[:], acc[it][:, OUT_DIM : OUT_DIM + 1])
                o = opool.tile([128, OUT_DIM], f32, tag="o")
                nc.vector.tensor_scalar_mul(o[:], acc[it][:, 0:OUT_DIM], rz[:])
                nc.sync.dma_start(out_d[it * 128 : (it + 1) * 128, :], o[:])

    nc.compile()
    return nc


def _prep_inputs(adj, x, fc_w, fc_b, attn_w, attn_b):
    fc_w = np.asarray(fc_w, np.float32)
    fc_b = np.asarray(fc_b, np.float32)
    attn_w = np.asarray(attn_w, np.float32)
    a_src = fc_w @ attn_w[:OUT_DIM]
    a_dst = fc_w @ attn_w[OUT_DIM:]
    b_src = float(fc_b @ attn_w[:OUT_DIM]) + float(attn_b)
    b_dst = float(fc_b @ attn_w[OUT_DIM:])

    xT = np.ascontiguousarray(np.asarray(x, np.float32).T).astype(bf16)
    adjT = np.asarray(adj, np.float32).astype(bf16).T  # [N (src j), N (dest i)]
    rhs_aug = np.concatenate(
        [fc_w, np.zeros((IN_DIM, 1), np.float32), a_dst[:, None]], axis=1
    ).astype(bf16)
    fcb_row = np.concatenate([fc_b, [1.0], [b_dst]]).astype(bf16)[None, :]
    w_src_rep = np.tile(a_src[:, None], (1, 128)).astype(bf16)
    src_bias = np.full((128, 1), b_src, np.float32)

    in_maps = []
    for c in range(NCORES):
        in_maps.append(
            {
                "adjT": np.ascontiguousarray(adjT[:, c * R : (c + 1) * R]),
                "xT": xT,
                "xTi": np.ascontiguousarray(xT[:, c * R : (c + 1) * R]),
                "rhs_aug": rhs_aug,
                "fcb_row": fcb_row,
                "w_src_rep": w_src_rep,
                "src_bias": src_bias,
            }
        )
    return in_maps


def kernel(adj, x, fc_w, fc_b, attn_w, attn_b, _trace=False, _tmpdir=None):
    from concourse import bass_utils

    if "nc" not in _cache:
        _cache["nc"] = _build()
    nc = _cache["nc"]
    in_maps = _prep_inputs(adj, x, fc_w, fc_b, attn_w, attn_b)
    res = bass_utils.run_bass_kernel_spmd(
        nc,
        in_maps,
        core_ids=list(range(NCORES)),
        trace=_trace,
        **({"tmpdir": _tmpdir} if _tmpdir else {}),
    )
    out = np.concatenate([res.results[c]["out"] for c in range(NCORES)], axis=0)
    if _trace:
        _cache["last_exec_time_ns"] = res.exec_time_ns
        _cache["last_profile_json"] = res.profile_json
    return out


# revision 18
# speedup vs baseline: 1.0986x; 1.0756x over previous
"""GAT layer (dense-adj variant) on 8 Trainium2 NeuronCores.

Strategy: row-parallel over destination nodes. Each core owns R=1024 rows of
the NxN score matrix / output; h (=x@fc_w+fc_b) is computed replicated on
every core. Scores are built in transposed layout [j (src) on partitions,
i (dest) on free] so the final attn@h matmul contracts j on partitions
directly. The softmax denominator Z rides along as column 256 of the moving
operand (h_aug's ones column), accumulated by the same matmuls as out.

Math (exact rank-1 decomposition of the reference):
  src = x@(fc_w@a_src) + (fc_b@a_src + attn_b)
  dst = x@(fc_w@a_dst) + (fc_b@a_dst)
  E[j,i] = exp(leaky_relu_{0.01}(src_i+dst_j) * adj[i,j])       (adj in {0,1})
  out[i,:] = (sum_j E[j,i] * h[j,:]) / (sum_j E[j,i])

Engine-level layout decisions (from NTFF traces):
- All elementwise data is bf16 (DVE 2x/4x modes; softmax rows are dominated
  by the 8191 exact exp(0)=1 non-edge terms per row, so bf16 score noise on
  the ~1% edges is invisible: emulated end-to-end rel err 3.2e-3 vs 3.0e-3
  for an all-f32 elementwise path).
- Per j-strip the E computation alternates between two equivalent forms to
  balance ScalarE vs VectorE:
    S1 (ACT-heavy): l = Prelu(src+dst) [ACT], za = l*adj [DVE], E = exp(za) [ACT]
    S2 (DVE-heavy): zb = src+dst [DVE], za = zb*adj [DVE], e1 = exp(za) [ACT],
                    t = 1+0.01*za [DVE], E = max(e1, t) [DVE]
  S2 uses exp(leaky(z)*adj) = exp(leaky(z*adj)) = max(exp(za), exp(0.01*za))
  with exp(0.01*za) ~ 1+0.01*za (error < 2e-3, exact at za=0 so non-edges
  stay exactly 1). Prelu/Exp share one ACT table set: no table reloads.
- fc_b/ones/b_dst enter h_aug through a 5th K=1 matmul (ones-row x fcb_row),
  so the PSUM->SBUF hop is a plain 2x-mode copy on DVE.
- One 8-bank PSUM pool: acc0..acc5 accumulate i-tiles 0..5 starting at strip
  0 (interleaved with phase B in the PE stream); banks 6/7 double as phase
  A/B scratch, so i-tiles 6/7 accumulate in a short tail after B finishes.
- Engines execute their instruction streams IN ORDER, so phase-B and phase-C
  work is emitted interleaved per 8-strip chunk; emitting all of B first
  starves ScalarE/TensorE until B completes.
"""

import numpy as np
import ml_dtypes

N = 8192
IN_DIM = 512
OUT_DIM = 256
NCORES = 8
R = N // NCORES  # 1024 rows per core
KT = IN_DIM // 128  # 4 k-tiles
JT = N // 128  # 64 j-strips
IT = R // 128  # 8 i-tiles per core
HA = OUT_DIM + 1  # h_aug matmul width (h | ones)
HS = OUT_DIM + 2  # h_sb slot width  (h | ones | dst)
GC = 8  # strips per emission chunk
N_EARLY = 6  # i-tiles accumulating from strip 0 (banks 0..5)

bf16 = ml_dtypes.bfloat16

_cache = {}

# Number of j-strips handled with the ACT-heavy form (S1); rest are S2.
N_S1 = 36


def _build():
    import concourse.tile as tile
    from concourse import bacc, mybir

    AF = mybir.ActivationFunctionType
    ALU = mybir.AluOpType
    f32 = mybir.dt.float32
    bft = mybir.dt.bfloat16

    s1_strips = set(np.linspace(0, JT - 1, N_S1).astype(int).tolist())

    nc = bacc.Bacc("TRN2", target_bir_lowering=False, debug=False)

    adjT_d = nc.dram_tensor("adjT", [N, R], bft, kind="ExternalInput").ap()
    xT_d = nc.dram_tensor("xT", [IN_DIM, N], bft, kind="ExternalInput").ap()
    xTi_d = nc.dram_tensor("xTi", [IN_DIM, R], bft, kind="ExternalInput").ap()
    # rhs_aug columns: [fc_w (256) | zeros (1) | w_dst (1)]
    rhs_aug_d = nc.dram_tensor("rhs_aug", [IN_DIM, HS], bft, kind="ExternalInput").ap()
    # fcb_aug columns: [fc_b replicated (256) | 1.0 | b_dst]
    fcb_aug_d = nc.dram_tensor("fcb_aug", [128, HS], f32, kind="ExternalInput").ap()
    w_src_rep_d = nc.dram_tensor("w_src_rep", [IN_DIM, 128], bft, kind="ExternalInput").ap()
    src_bias_d = nc.dram_tensor("src_bias", [128, 1], f32, kind="ExternalInput").ap()
    out_d = nc.dram_tensor("out", [R, OUT_DIM], f32, kind="ExternalOutput").ap()

    with tile.TileContext(nc) as tc:
        with (
            tc.tile_pool(name="const", bufs=1) as cpool,
            tc.tile_pool(name="hpool", bufs=1) as hpool,
            tc.tile_pool(name="xstream", bufs=8) as xpool,
            tc.tile_pool(name="astream", bufs=8) as apool,
            tc.tile_pool(name="work", bufs=3) as wpool,
            tc.tile_pool(name="estream", bufs=24) as epool,
            tc.tile_pool(name="opool", bufs=2) as opool,
        ):
            # ---- constants ----
            rhs_aug_sb = cpool.tile([128, KT * HS], bft)
            nc.sync.dma_start(
                rhs_aug_sb[:].rearrange("p (k n) -> p k n", k=KT),
                rhs_aug_d.rearrange("(k p) n -> p k n", p=128),
            )
            fcb_aug_sb = cpool.tile([128, HS], f32)
            nc.sync.dma_start(fcb_aug_sb[:], fcb_aug_d)
            w_src_sb = cpool.tile([128, KT * 128], bft)
            nc.sync.dma_start(
                w_src_sb[:].rearrange("p (k n) -> p k n", k=KT),
                w_src_rep_d.rearrange("(k p) n -> p k n", p=128),
            )
            xTi_sb = cpool.tile([128, KT * R], bft)
            nc.sync.dma_start(
                xTi_sb[:].rearrange("p (k n) -> p k n", k=KT),
                xTi_d.rearrange("(k p) n -> p k n", p=128),
            )
            src_bias_sb = cpool.tile([128, 1], f32)
            nc.sync.dma_start(src_bias_sb[:], src_bias_d)

            src_rep = cpool.tile([128, R], bft)
            h_sb = hpool.tile([128, JT * HS], bft)
            dst_sb = cpool.tile([128, JT], f32)
            e_strips = [None] * JT

            def c_elementwise(jt):
                # E[j,i] strip for one 128-node j block (see module docstring)
                adjt = apool.tile([128, R], bft, name="adjt")
                nc.sync.dma_start(adjt[:], adjT_d[jt * 128 : (jt + 1) * 128, :])
                dst_j = dst_sb[:, jt : jt + 1]  # f32 [128,1]
                e = epool.tile([128, R], bft, name="e")
                if jt in s1_strips:
                    l = wpool.tile([128, R], bft, name="l", tag="l")
                    nc.scalar.activation(
                        l[:], src_rep[:], AF.Prelu, bias=dst_j, alpha=0.01,
                    )
                    za = wpool.tile([128, R], bft, name="za", tag="za")
                    nc.vector.tensor_mul(za[:], l[:], adjt[:])
                    nc.scalar.activation(e[:], za[:], AF.Exp)
                else:
                    zb = wpool.tile([128, R], bft, name="zb", tag="zb")
                    nc.vector.tensor_scalar_add(zb[:], src_rep[:], dst_j)
                    za = wpool.tile([128, R], bft, name="za", tag="za")
                    nc.vector.tensor_mul(za[:], zb[:], adjt[:])
                    e1 = wpool.tile([128, R], bft, name="e1", tag="e1")
                    nc.scalar.activation(e1[:], za[:], AF.Exp)
                    t = wpool.tile([128, R], bft, name="t", tag="t")
                    nc.vector.tensor_scalar(
                        t[:], za[:], 0.01, 1.0, ALU.mult, ALU.add,
                    )
                    nc.vector.tensor_max(e[:], e1[:], t[:])
                e_strips[jt] = e

            out_ps = {}

            def c_matmuls(jt, its):
                e = e_strips[jt]
                hj = h_sb[:, jt * HS : jt * HS + HA]
                for it in its:
                    nc.tensor.matmul(
                        out_ps[it][:, 0:HA],
                        e[:, it * 128 : (it + 1) * 128],
                        hj,
                        start=(jt == 0),
                        stop=(jt == JT - 1),
                    )

            ps_ab_cm = tc.tile_pool(name="ps_ab", bufs=4, space="PSUM")
            ps_ab = ps_ab_cm.__enter__()
            # ---- Phase A: src_rep[p, f] = src[i0+f] for all p ----
            for ch in range(R // 512):
                ps = ps_ab.tile([128, 512], f32, name="ps_a", tag="ps")
                for kt in range(KT):
                    nc.tensor.matmul(
                        ps[:],
                        w_src_sb[:, kt * 128 : (kt + 1) * 128],
                        xTi_sb[:, kt * R + ch * 512 : kt * R + (ch + 1) * 512],
                        start=(kt == 0),
                        stop=(kt == KT - 1),
                    )
                nc.scalar.activation(
                    src_rep[:, ch * 512 : (ch + 1) * 512], ps[:], AF.Identity,
                    bias=src_bias_sb[:],
                )

            # ---- Phases B + C interleaved per chunk ----
            for jt in range(JT):
                xTj = xpool.tile([128, KT * 128], bft)
                nc.sync.dma_start(
                    xTj[:].rearrange("p (k n) -> p k n", k=KT),
                    xT_d[:, jt * 128 : (jt + 1) * 128].rearrange(
                        "(k p) n -> p k n", p=128
                    ),
                )
                ps = ps_ab.tile([128, 512], f32, name="ps_b", tag="ps")
                for kt in range(KT):
                    nc.tensor.matmul(
                        ps[:, 0:HS],
                        xTj[:, kt * 128 : (kt + 1) * 128],
                        rhs_aug_sb[:, kt * HS : (kt + 1) * HS],
                        start=(kt == 0),
                        stop=(kt == KT - 1),
                    )
                # slot: [h+fc_b (256) | 1.0 (0+1) | dst+b_dst]
                nc.vector.tensor_add(
                    h_sb[:, jt * HS : (jt + 1) * HS], ps[:, 0:HS], fcb_aug_sb[:],
                )
                if jt % GC == GC - 1:
                    g = jt // GC
                    nc.vector.tensor_copy(
                        dst_sb[:, g * GC : (g + 1) * GC],
                        h_sb[:, g * GC * HS : (g + 1) * GC * HS].rearrange(
                            "p (j s) -> p j s", s=HS
                        )[:, :, HS - 1 : HS],
                    )
                    for s_jt in range(g * GC, (g + 1) * GC):
                        c_elementwise(s_jt)

            # ---- Phase C matmuls: 8 PSUM banks after A/B's pool closes ----
            ps_ab_cm.__exit__(None, None, None)
            with tc.tile_pool(name="ps_acc", bufs=1, space="PSUM") as ps_acc:
                for it in range(IT):
                    out_ps[it] = ps_acc.tile(
                        [128, HA], f32, name=f"acc{it}", tag=f"acc{it}"
                    )
                for jt in range(JT):
                    c_matmuls(jt, range(IT))

                # ---- Phase D: normalize rows (col 256 = Z) and store ----
                for it in range(IT):
                    rz = opool.tile([128, 1], f32, tag="rz")
                    nc.vector.reciprocal(rz[:], out_ps[it][:, OUT_DIM : OUT_DIM + 1])
                    o = opool.tile([128, OUT_DIM], f32, tag="o")
                    nc.vector.tensor_scalar_mul(o[:], out_ps[it][:, 0:OUT_DIM], rz[:])
                    nc.sync.dma_start(out_d[it * 128 : (it + 1) * 128, :], o[:])

    nc.compile()
    return nc


def _prep_inputs(adj, x, fc_w, fc_b, attn_w, attn_b):
    fc_w = np.asarray(fc_w, np.float32)
    fc_b = np.asarray(fc_b, np.float32)
    attn_w = np.asarray(attn_w, np.float32)
    a_src = fc_w @ attn_w[:OUT_DIM]
    a_dst = fc_w @ attn_w[OUT_DIM:]
    b_src = float(fc_b @ attn_w[:OUT_DIM]) + float(attn_b)
    b_dst = float(fc_b @ attn_w[OUT_DIM:])

    xT = np.ascontiguousarray(np.asarray(x, np.float32).T).astype(bf16)
    adjT = np.asarray(adj, np.float32).astype(bf16).T  # [N (src j), N (dest i)]
    rhs_aug = np.concatenate(
        [fc_w, np.zeros((IN_DIM, 1), np.float32), a_dst[:, None]], axis=1
    ).astype(bf16)
    fcb_aug = np.concatenate(
        [
            np.tile(fc_b[None, :], (128, 1)),
            np.ones((128, 1), np.float32),
            np.full((128, 1), b_dst, np.float32),
        ],
        axis=1,
    ).astype(np.float32)
    w_src_rep = np.tile(a_src[:, None], (1, 128)).astype(bf16)
    src_bias = np.full((128, 1), b_src, np.float32)

    in_maps = []
    for c in range(NCORES):
        in_maps.append(
            {
                "adjT": np.ascontiguousarray(adjT[:, c * R : (c + 1) * R]),
                "xT": xT,
                "xTi": np.ascontiguousarray(xT[:, c * R : (c + 1) * R]),
                "rhs_aug": rhs_aug,
                "fcb_aug": fcb_aug,
                "w_src_rep": w_src_rep,
                "src_bias": src_bias,
            }
        )
    return in_maps


def kernel(adj, x, fc_w, fc_b, attn_w, attn_b, _trace=False, _tmpdir=None):
    from concourse import bass_utils

    if "nc" not in _cache:
        _cache["nc"] = _build()
    nc = _cache["nc"]
    in_maps = _prep_inputs(adj, x, fc_w, fc_b, attn_w, attn_b)
    res = bass_utils.run_bass_kernel_spmd(
        nc,
        in_maps,
        core_ids=list(range(NCORES)),
        trace=_trace,
        **({"tmpdir": _tmpdir} if _tmpdir else {}),
    )
    out = np.concatenate([res.results[c]["out"] for c in range(NCORES)], axis=0)
    if _trace:
        _cache["last_exec_time_ns"] = res.exec_time_ns
        _cache["last_profile_json"] = res.profile_json
    return out
